# revision 19
# baseline (speedup 1.0000x reference)
"""Trainium2 Bass kernel for nn_CRF_70239895159020.

Reference (B=524288, C=70, 10 iterations):
    L = (S + S^T)/2 ; dL = diag(L) ; Q = log_softmax(logits, axis=1)
    repeat 10x:  P = 2*exp(Q)-1
                 tmp = logits + P @ L - dL*P       (L symmetric)
                 Q = log_sigmoid(2*tmp)

Reformulation (M = L with zero diagonal, c = colsum(M)):
    tmp2 := 2*tmp = 2*(logits + E @ M) - 2*c   with E := exp(Q)
    E_{t+1} = sigmoid(tmp2_t),  E_0 = softmax(logits)
    output  = log_sigmoid(tmp2_9) = -ln(1 + exp(-tmp2_9))

Device layout: state transposed, E^T in SBUF [C=70, ncols]; per
512-column block PSUM accumulates E@M2 + logits^T via matmuls (the
logits add uses identity weights), then one ScalarE pass computes
sigmoid(2*psum + bias) with per-partition bias = -2c.

Precision: the fixed-point iteration amplifies matmul noise ~100x
(l2) / ~1e5x (worst element), so reduced-precision inputs in EARLY
iterations are fatal while late iterations tolerate fp16.  Hence:
  - iterations t < K_HI: E kept fp32, E-matmul in fp32 (4 cyc/row),
    logits^T added as an fp16 hi+lo pair (2 matmuls, exact to 2^-21)
  - iterations t >= K_HI: E in fp16 (ScalarE writes fp16 directly),
    fp16 matmuls at 1 cyc/row, logits^T hi part only
The logits^T hi/lo pair is split on the host and DMA-transposed
directly into SBUF (2-byte DMA transpose), so no PE transposes or
DVE copies are spent on it.  The lo-part add runs on VectorE in the
early (PE-bound) iterations.

DMA layout: batch rows map to (partition, tile) as row = p*TPC + t so
every partition's load/store is one contiguous ~18KB block (128 fat
descriptors per chunk instead of 8192 280-byte rows).  The transposed
domain inherits a fixed column permutation, which is harmless (all
math is per-column) as long as logits^T is permuted to match — done
on the host when splitting hi/lo.

Final iteration: x := tmp2_9; save -x (fp16); u = exp(-x) in PSUM;
clamp u to 1e15 (HW Ln input range is +-2^64); v = ln(1+u) (fp16);
q = max(v, -x) recovers the clamped region exactly (|x|>34 there, so
log_sigmoid(x) = x to fp32); output = -q^T via PE transpose.

Sharding: batch split 8 ways across cores (pure data parallel).

Measured operating point (HW-verified, cost-model timing): l2 rel err
7.37e-4, absmax 0.61 (1.04% of output scale), ~1.40 ms/core.  PE busy
is 1083 us against a ~1030 us floor for this precision schedule.
KERNEL_K_HI=4 trades absmax 0.61 -> 0.85 for ~100 us if a faster,
slightly less accurate point is ever preferred.
"""

import os
import numpy as np

B = 524288
C = 70
N_CORES = 8
B_CORE = B // N_CORES
ITERS = 10

# tunables
NCH = 8192            # batch columns per chunk (transposed free dim)
BLK = 2048            # psum block columns (4 banks)
K_HI = int(os.environ.get("KERNEL_K_HI", "5"))  # exact early iterations
L1_DMAT = os.environ.get("KERNEL_L1_DMAT", "1") == "1"  # dma-transpose L1 pair

_prog_cache = {}
LAST_RESULTS = None


def build_program(b_core=B_CORE, nch=NCH, blk=BLK, k_hi=K_HI,
                  l1_dmat=L1_DMAT):
    import concourse.bass as bass
    import concourse.bacc as bacc
    import concourse.tile as tile
    from concourse import mybir
    from contextlib import ExitStack

    f32 = mybir.dt.float32
    f16 = mybir.dt.float16
    AF = mybir.ActivationFunctionType
    Alu = mybir.AluOpType

    assert b_core % nch == 0
    nchunks = b_core // nch
    tpc = nch // 128
    assert nch % blk == 0 and blk % 512 == 0
    nblk = nch // blk
    gfwd = blk // 128          # fwd transposes per psum group
    gbwd = 4                   # natural slices per bwd psum group
    assert tpc % gbwd == 0

    nc = bacc.Bacc("TRN2", target_bir_lowering=False)

    logits_d = nc.dram_tensor("logits", [b_core, C], f32, kind="ExternalInput")
    if l1_dmat:
        lhi_d = nc.dram_tensor("lhi", [b_core, C], f16, kind="ExternalInput")
        llo_d = nc.dram_tensor("llo", [b_core, C], f16, kind="ExternalInput")
    cf32_d = nc.dram_tensor("cf32", [128, 200], f32, kind="ExternalInput")
    cf16_d = nc.dram_tensor("cf16", [C, 140], f16, kind="ExternalInput")
    out_d = nc.dram_tensor("out", [b_core, C], f32, kind="ExternalOutput")

    # row = k*nch + p*tpc + t: each partition reads/writes one contiguous
    # tpc*C*4B block per chunk (128 fat DMA descriptors instead of 8192
    # 280-byte rows).  The transposed-domain column order becomes
    # p-major, which is fine: the computation is independent per column
    # and E0/L1^T/output all inherit the same permutation.
    lg = logits_d[:, :].rearrange("(k p t) c -> k p t c", p=128, t=tpc)
    og = out_d[:, :].rearrange("(k p t) c -> k p t c", p=128, t=tpc)

    with tile.TileContext(nc) as tc, ExitStack() as ctx:
        const = ctx.enter_context(tc.tile_pool(name="const", bufs=1))
        natp = ctx.enter_context(tc.tile_pool(name="nat", bufs=2))
        stagp = ctx.enter_context(tc.tile_pool(name="stag", bufs=1))
        e32p = ctx.enter_context(tc.tile_pool(name="e32", bufs=1))
        e16p = ctx.enter_context(tc.tile_pool(name="e16", bufs=1))
        l1p = ctx.enter_context(tc.tile_pool(name="l1", bufs=2))
        smallp = ctx.enter_context(tc.tile_pool(name="small", bufs=1))
        psp = ctx.enter_context(tc.tile_pool(name="ps", bufs=2, space="PSUM"))

        cf32 = const.tile([128, 200], f32)
        nc.sync.dma_start(out=cf32, in_=cf32_d[:, :])
        cf16 = const.tile([C, 140], f16)
        nc.sync.dma_start(out=cf16, in_=cf16_d[:, :])
        ident = cf32[:, 0:128]
        m2sb = cf32[:C, 128:128 + C]
        b2sb = cf32[:C, 198:199]
        b2nsb = cf32[:C, 199:200]
        m2h = cf16[:, 0:C]
        idh = cf16[:, C:2 * C]
        tc.strict_bb_all_engine_barrier()

        def phase_a(k):
            rows = slice(k * nch, (k + 1) * nch)
            natk = natp.tile([128, tpc, C], f32, tag="nat")
            nc.sync.dma_start(out=natk, in_=lg[k])
            l1hi = l1p.tile([C, nch], f16, tag="Lhi")
            l1lo = l1p.tile([C, nch], f16, tag="Llo")
            if l1_dmat:
                nc.sync.dma_start_transpose(out=l1hi, in_=lhi_d[rows, :])
                nc.sync.dma_start_transpose(out=l1lo, in_=llo_d[rows, :])
            else:
                for g in range(tpc // gfwd):
                    pt = psp.tile([C, gfwd * 128], f32, tag="ps")
                    for s in range(gfwd):
                        t = g * gfwd + s
                        nc.tensor.transpose(
                            pt[:, s * 128:(s + 1) * 128], natk[:, t, :], ident)
                    gsl = slice(g * gfwd * 128, (g + 1) * gfwd * 128)
                    nc.vector.tensor_copy(out=l1hi[:, gsl], in_=pt)
                    nc.vector.tensor_sub(out=l1lo[:, gsl], in0=pt,
                                         in1=l1hi[:, gsl])

            ek32 = e32p.tile([C, nch], f32, tag="E32")
            ek16 = e16p.tile([C, nch], f16, tag="E16")
            xn16 = e16p.tile([C, nch], f16, tag="XN")

            # softmax in natural layout (in place on natk)
            nc.scalar.activation(natk, natk, AF.Exp)
            s_t = smallp.tile([128, tpc], f32, tag="s")
            nc.vector.reduce_sum(out=s_t, in_=natk, axis=mybir.AxisListType.X)
            r_t = smallp.tile([128, tpc], f32, tag="r")
            nc.vector.reciprocal(out=r_t, in_=s_t)
            t1 = smallp.tile([128, tpc], f32, tag="t1")
            nc.vector.tensor_mul(out=t1, in0=s_t, in1=r_t)
            nc.vector.tensor_scalar(out=t1, in0=t1, scalar1=-1.0, scalar2=2.0,
                                    op0=Alu.mult, op1=Alu.add)
            nc.vector.tensor_mul(out=r_t, in0=r_t, in1=t1)
            import concourse.bass as _b
            r_bcast = _b.AP(
                tensor=r_t.tensor, offset=r_t.offset,
                ap=[r_t.ap[0], r_t.ap[1], [0, C]])
            nc.vector.tensor_mul(out=natk, in0=natk, in1=r_bcast)
            # E0^T via PE transposes -> ek32 (or ek16 when k_hi == 0)
            e0dst = ek32 if k_hi > 0 else ek16
            for g in range(tpc // gfwd):
                pt = psp.tile([C, gfwd * 128], f32, tag="ps")
                for s in range(gfwd):
                    t = g * gfwd + s
                    nc.tensor.transpose(
                        pt[:, s * 128:(s + 1) * 128], natk[:, t, :], ident)
                nc.vector.tensor_copy(
                    out=e0dst[:, g * gfwd * 128:(g + 1) * gfwd * 128], in_=pt)
            return l1hi, l1lo, ek32, ek16, xn16

        def phase_b(tiles):
            l1hi, l1lo, ek32, ek16, xn16 = tiles
            for it in range(ITERS):
                hi = it < k_hi
                last = it == ITERS - 1
                for j in range(nblk):
                    pt = psp.tile([C, blk], f32, tag="ps")
                    for q in range(blk // 512):
                        lo = j * blk + q * 512
                        sub = pt[:, q * 512:(q + 1) * 512]
                        if hi:
                            nc.tensor.matmul(sub, lhsT=m2sb,
                                             rhs=ek32[:, lo:lo + 512],
                                             start=True, stop=False)
                            nc.tensor.matmul(sub, lhsT=idh,
                                             rhs=l1hi[:, lo:lo + 512],
                                             start=False, stop=True)
                        else:
                            nc.tensor.matmul(sub, lhsT=m2h,
                                             rhs=ek16[:, lo:lo + 512],
                                             start=True, stop=False)
                            nc.tensor.matmul(sub, lhsT=idh,
                                             rhs=l1hi[:, lo:lo + 512],
                                             start=False, stop=True)
                    jsl = slice(j * blk, (j + 1) * blk)
                    if hi:
                        # l1lo correction added on VectorE (PE-bound phase)
                        nc.vector.tensor_add(out=pt, in0=pt, in1=l1lo[:, jsl])
                    if not last:
                        dst = ek32 if (it + 1 < k_hi) else ek16
                        nc.scalar.activation(dst[:, jsl], pt, AF.Sigmoid,
                                             bias=b2sb, scale=2.0)
                    else:
                        # x = 2*psum + b2 ; save -x (fp16) and u = exp(-x)
                        # (fp32, into the dead ek32) per block
                        nc.vector.tensor_scalar(
                            out=xn16[:, jsl], in0=pt,
                            scalar1=-2.0, scalar2=b2nsb,
                            op0=Alu.mult, op1=Alu.add)
                        nc.scalar.activation(ek32[:, jsl], pt, AF.Exp,
                                             bias=b2nsb, scale=-2.0)

        def phase_c(k, tiles):
            l1hi, l1lo, ek32, ek16, xn16 = tiles
            # v = ln(1+min(u,1e15)); q = max(v,-x); out = -q^T.
            # Block-wise so the DVE/ACT/PE tail pipelines instead of three
            # whole-chunk serial passes (ln stays grouped after all exps,
            # so ACT table sets do not thrash).
            stagk = stagp.tile([128, tpc, C], f32, tag="stag")
            gpb = blk // (gbwd * 128)      # bwd transpose groups per block
            for j in range(nblk):
                jsl = slice(j * blk, (j + 1) * blk)
                nc.vector.tensor_scalar_min(out=ek32[:, jsl], in0=ek32[:, jsl],
                                            scalar1=1e15)
                nc.scalar.activation(ek16[:, jsl], ek32[:, jsl], AF.Ln,
                                     bias=1.0, scale=1.0)
                nc.vector.tensor_max(out=ek16[:, jsl], in0=ek16[:, jsl],
                                     in1=xn16[:, jsl])
                for gg in range(gpb):
                    g = j * gpb + gg
                    pn = psp.tile([128, gbwd * C], f16, tag="ps")
                    for s in range(gbwd):
                        t = g * gbwd + s
                        nc.tensor.transpose(
                            pn[:, s * C:(s + 1) * C],
                            ek16[:, t * 128:(t + 1) * 128], idh)
                    nc.vector.tensor_scalar_mul(
                        out=stagk[:, g * gbwd:(g + 1) * gbwd, :],
                        in0=pn.rearrange("p (a c) -> p a c", c=C),
                        scalar1=-1.0)
            nc.sync.dma_start(out=og[k], in_=stagk)

        # software-pipelined trace order: chunk k+1's phase A is emitted
        # before chunk k's tail so PE has work across the boundary
        tiles = phase_a(0)
        for k in range(nchunks):
            phase_b(tiles)
            nxt = phase_a(k + 1) if k + 1 < nchunks else None
            phase_c(k, tiles)
            tiles = nxt

    nc.compile()
    return nc


def _host_prep(logits, similarities):
    S = np.asarray(similarities, dtype=np.float32)
    L = (S + S.T) * np.float32(0.5)
    M = L.copy()
    np.fill_diagonal(M, 0.0)
    m2 = (2.0 * M).astype(np.float32)
    m2h = m2.astype(np.float16)
    col = M.astype(np.float64).sum(axis=0)
    cf32 = np.zeros((128, 200), dtype=np.float32)
    cf32[:, 0:128] = np.eye(128, dtype=np.float32)
    cf32[:C, 128:128 + C] = m2
    cf32[:C, 198] = (-2.0 * col).astype(np.float32)
    cf32[:C, 199] = (2.0 * col).astype(np.float32)
    cf16 = np.zeros((C, 140), dtype=np.float16)
    cf16[:, 0:C] = m2h
    cf16[:, C:2 * C] = np.eye(C, dtype=np.float16)
    lhi = logits.astype(np.float16)
    llo = (logits - lhi.astype(np.float32)).astype(np.float16)
    if L1_DMAT:
        # device ek column n (of chunk k) holds batch row k*NCH + p*TPC + t
        # where n = t*128 + p; reorder so dma_start_transpose's row-major
        # fill matches: row' (t*128+p) <- row (p*TPC+t)
        tpc = NCH // 128

        def perm(a):
            b_all, c = a.shape
            v = a.reshape(b_all // NCH, 128, tpc, c)       # [k, p, t, c]
            v = np.ascontiguousarray(v.transpose(0, 2, 1, 3))  # [k, t, p, c]
            return v.reshape(b_all, c)
        lhi = perm(lhi)
        llo = perm(llo)
    return cf32, cf16, lhi, llo


def kernel(logits, similarities):
    global LAST_RESULTS
    from concourse.bass_utils import run_bass_kernel_spmd

    logits = np.ascontiguousarray(np.asarray(logits), dtype=np.float32)
    cf32, cf16, lhi, llo = _host_prep(logits, similarities)

    key = (B_CORE, NCH, BLK, K_HI, L1_DMAT)
    if key not in _prog_cache:
        _prog_cache[key] = build_program()
    nc = _prog_cache[key]

    shards = logits.reshape(N_CORES, B_CORE, C)
    lhi_s = lhi.reshape(N_CORES, B_CORE, C)
    llo_s = llo.reshape(N_CORES, B_CORE, C)
    in_maps = []
    for i in range(N_CORES):
        m = {"logits": shards[i], "cf32": cf32, "cf16": cf16}
        if L1_DMAT:
            m["lhi"] = np.ascontiguousarray(lhi_s[i])
            m["llo"] = np.ascontiguousarray(llo_s[i])
        in_maps.append(m)
    trace = os.environ.get("KERNEL_TRACE", "0") == "1"
    res = run_bass_kernel_spmd(nc, in_maps, core_ids=list(range(N_CORES)),
                               trace=trace)
    LAST_RESULTS = res
    out = np.concatenate([r["out"] for r in res.results], axis=0)
    return np.ascontiguousarray(out, dtype=np.float32)
